# revision 7
# baseline (speedup 1.0000x reference)
"""Trainium2 Bass kernel for MoEResNetBKLayer.

Strategy (8 NeuronCores, SPMD). The dominant cost in this harness is the
axon-tunneled host->device transfer (~33MB/s), so the kernel is built to
minimize bytes shipped per dispatch while keeping the real compute
(expert FFN matmuls, BK tridiagonal scan, spec projection) on device:

  - Host: top-1 routing (argmax of gate logits), sort tokens by expert.
    Core c handles expert c//2, token-half c%2, capacity 576 slots
    (per-expert capacity 1152 >> binomial(4096, 1/4) tail; host fallback
    if ever exceeded).
  - Weights: each core ships only HALF of its expert's w1/w2 (4MB+4MB
    bf16); the full expert weights are assembled on device by a pairwise
    AllGather over NeuronLink. Cuts weight traffic 128MB -> 64MB.
  - BK spectral branch: host computes the trivial potential matvec
    v = clip(x@v_w+v_b) (replaces shipping full x, 64MB -> 16KB/core);
    device runs the blocked Mobius/continued-fraction scan: 32-step
    within-block 3-term recurrences on 128 lanes, cross-block scan,
    vectorized application -> G diag (complex) for all 4096 tokens.
  - G gathered to this core's slots via one-hot matmul; the one-hot is
    built ON DEVICE from shipped token ids (replaces the 5MB/core
    one-hot matrix), then folded into the MM2 PSUM via a rank-2 matmul
    with W' = bk_scale*out_w; bias (b2 + bk*out_b) added on output copy.
  - Routed expert FFN on gathered tokens: h = gelu(x_g @ w1.T + b1),
    y = h @ w2.T (bf16 matmuls, fp32 PSUM accum). Output shipped fp16.
  - Host: scatter per-slot outputs back to token order (pure indexing).
"""

import sys as _sys
for _p in ("/opt/trn_rl_repo",):
    if _p not in _sys.path:
        _sys.path.append(_p)
import numpy as np
import ml_dtypes

B, N, D, E, F = 2, 2048, 1024, 4, 4096
NT = B * N              # 4096 tokens
KS = 32                 # scan block size (steps)
NBLK = N // KS          # 64 blocks per row
LANES = B * NBLK        # 128
CAP = 576               # token slots per core
FH = F // 2             # expert F-half per core
NC = 8                  # cores
SUP = 8                 # superblocks in cross-block scan (8 x 8 = 64)
V_MAX = 3.0
FCLAMP = 10.0

bf16 = ml_dtypes.bfloat16

_PROG_CACHE = {}
_LAST_IN_MAPS = None

PAIRS = [[0, 1], [2, 3], [4, 5], [6, 7]]


def _build_program():
    import concourse.bass as bass
    import concourse.tile as tile
    from concourse import bacc, mybir

    fp32 = mybir.dt.float32
    fp16 = mybir.dt.float16
    bfl = mybir.dt.bfloat16
    AF = mybir.ActivationFunctionType
    OP = mybir.AluOpType

    nc = bacc.Bacc("TRN2", target_bir_lowering=False, debug=False, num_devices=NC)

    def din(name, shape, dt):
        return nc.dram_tensor(name, list(shape), dt, kind="ExternalInput").ap()

    xgt = din("xgt", (D, CAP), bfl)         # gathered tokens, transposed
    w1h = din("w1h", (D // 2, F), bfl)      # this core's row-half of w1[e].T
    w2h = din("w2h", (FH, D), bfl)          # this core's row-half of w2[e].T
    waug = din("waug", (2, D), bfl)         # [bk*out_w[:,0]; bk*out_w[:,1]]
    he = din("he", (128, KS), fp32)         # clip(x@v_w+v_b,+-3)-2, lane layout
    dimt = din("dimt", (128, KS), fp32)     # -(eps+gamma) everywhere
    cfirst = din("cfirst", (128, 1), fp32)  # 0 where lane%64==0 else 1
    clast = din("clast", (128, 1), fp32)    # 0 where lane%64==63 else 1
    iotac = din("iotac", (128, 1), fp32)    # 0..127 per partition
    tokb = din("tokb", (128, CAP), fp32)    # token id per slot, bcast over parts
    b1t = din("b1t", (128, F // 128), fp32)  # b1[e] chunk-major
    ballt = din("ballt", (128, D // 128), fp32)  # b2[e]+bk*out_b chunk-major

    outg = nc.dram_tensor("outg", [D, CAP], fp16, kind="ExternalOutput").ap()

    # device-side gathered weights (pairwise AllGather). Row-halves
    # concatenate back to the full contiguous w1[e].T / w2[e].T layouts.
    locw1 = nc.dram_tensor("locw1", [D // 2, F], bfl, kind="Internal").ap()
    locw2 = nc.dram_tensor("locw2", [FH, D], bfl, kind="Internal").ap()
    w1g = nc.dram_tensor("w1g", [D, F], bfl, kind="Internal").ap()
    w2g = nc.dram_tensor("w2g", [F, D], bfl, kind="Internal").ap()

    FCH = F // 128   # 32
    DCH = D // 128   # 8
    NCH = [(0, 512), (512, CAP - 512)]  # CAP split for PSUM banks

    from contextlib import ExitStack

    with tile.TileContext(nc) as tc, ExitStack() as ctx:
        const_p = ctx.enter_context(tc.tile_pool(name="const", bufs=1))
        dram_p = ctx.enter_context(tc.tile_pool(name="dram", bufs=1, space="DRAM"))
        xin_p = ctx.enter_context(tc.tile_pool(name="xin", bufs=3))
        w_p = ctx.enter_context(tc.tile_pool(name="w", bufs=2))
        p_p = ctx.enter_context(tc.tile_pool(name="p", bufs=3))
        big_p = ctx.enter_context(tc.tile_pool(name="big", bufs=1))
        scan_p = ctx.enter_context(tc.tile_pool(name="scan", bufs=1))
        ps_mm = ctx.enter_context(tc.tile_pool(name="psmm", bufs=2, space="PSUM"))
        ps_g = ctx.enter_context(tc.tile_pool(name="psg", bufs=1, space="PSUM"))

        # ---- weight halves -> internal DRAM -> pairwise AllGather ----
        nc.sync.dma_start(locw1[:], w1h[:])
        nc.sync.dma_start(locw2[:], w2h[:])
        nc.gpsimd.collective_compute(
            "AllGather", OP.bypass, PAIRS, [locw1[:]], [w1g[:]])
        nc.gpsimd.collective_compute(
            "AllGather", OP.bypass, PAIRS, [locw2[:]], [w2g[:]])

        # ---- constants to SBUF ----
        dim_s = const_p.tile([128, KS], fp32)
        nc.sync.dma_start(dim_s[:], dimt[:])
        cf_s = const_p.tile([128, 1], fp32)
        nc.sync.dma_start(cf_s[:], cfirst[:])
        cl_s = const_p.tile([128, 1], fp32)
        nc.sync.dma_start(cl_s[:], clast[:])
        io_s = const_p.tile([128, 1], fp32)
        nc.sync.dma_start(io_s[:], iotac[:])
        tokb_s = const_p.tile([128, CAP], fp32)
        nc.sync.dma_start(tokb_s[:], tokb[:])
        b1_s = const_p.tile([128, FCH], fp32)
        nc.sync.dma_start(b1_s[:], b1t[:])
        ball_s = const_p.tile([128, DCH], fp32)
        nc.sync.dma_start(ball_s[:], ballt[:])
        waug_s = const_p.tile([2, D], bfl)
        nc.sync.dma_start(waug_s[:], waug[:])

        # ---- DRAM scratch for scan bounces ----
        grd = dram_p.tile([128, KS], bfl)       # G.real token order
        gid = dram_p.tile([128, KS], bfl)
        cbd = dram_p.tile([16, 128], fp32)      # block-matrix bounce
        lcd = dram_p.tile([4, 128], fp32)       # carries bounce

        # ================= BK scan =================
        he_s = scan_p.tile([128, KS], fp32, tag="he")
        nc.sync.dma_start(he_s[:], he[:])
        he = he_s  # alias: rest of scan uses the tile

        # ============ within-block 3-term recurrences ============
        # fwd arrays (128, 2*(KS+2)): [ar | br] re-part, [ai | bi] im-part
        W2 = KS + 2
        fr = scan_p.tile([128, 2 * W2], fp32, tag="fr")
        fi = scan_p.tile([128, 2 * W2], fp32, tag="fi")
        br_ = scan_p.tile([128, 2 * W2], fp32, tag="br")
        bi_ = scan_p.tile([128, 2 * W2], fp32, tag="bi")
        tmp2 = scan_p.tile([128, 2], fp32, tag="tmp2")

        def pair(tile_, c):  # columns {c, W2+c} as (128,2) strided AP
            return tile_.rearrange("p (x c) -> p c x", x=2)[:, c, :]

        # seeds fwd: a_{-2}=0,a_{-1}=1 ; b_{-2}=cfirst, b_{-1}=0
        nc.gpsimd.memset(fr[:, 0:2], 0.0)
        nc.gpsimd.memset(fr[:, W2:W2 + 2], 0.0)
        nc.vector.tensor_scalar_add(fr[:, 1:2], fr[:, 1:2], 1.0)
        nc.vector.tensor_copy(fr[:, W2:W2 + 1], cf_s[:])
        nc.gpsimd.memset(fi[:, 0:2], 0.0)
        nc.gpsimd.memset(fi[:, W2:W2 + 2], 0.0)
        # seeds bwd: a_{K}=1,a_{K+1}=0 ; b_{K}=0, b_{K+1}=clast
        nc.gpsimd.memset(br_[:, KS:KS + 2], 0.0)
        nc.gpsimd.memset(br_[:, W2 + KS:W2 + KS + 2], 0.0)
        nc.vector.tensor_scalar_add(br_[:, KS:KS + 1], br_[:, KS:KS + 1], 1.0)
        nc.vector.tensor_copy(br_[:, W2 + KS + 1:W2 + KS + 2], cl_s[:])
        nc.gpsimd.memset(bi_[:, KS:KS + 2], 0.0)
        nc.gpsimd.memset(bi_[:, W2 + KS:W2 + KS + 2], 0.0)

        di0 = dim_s[:, 0:1]
        for s in range(KS):
            drs = he[:, s:s + 1]
            # re: new = dr*prev_r - di*prev_i - prev2_r
            nc.vector.scalar_tensor_tensor(
                tmp2[:], pair(fi, s + 1), di0, pair(fr, s), OP.mult, OP.add)
            nc.vector.scalar_tensor_tensor(
                pair(fr, s + 2), pair(fr, s + 1), drs, tmp2[:], OP.mult, OP.subtract)
            # im: new = dr*prev_i + di*prev_r - prev2_i
            nc.vector.scalar_tensor_tensor(
                tmp2[:], pair(fr, s + 1), di0, pair(fi, s), OP.mult, OP.subtract)
            nc.vector.scalar_tensor_tensor(
                pair(fi, s + 2), pair(fi, s + 1), drs, tmp2[:], OP.mult, OP.add)
        for s in range(KS - 1, -1, -1):
            drs = he[:, s:s + 1]
            nc.vector.scalar_tensor_tensor(
                tmp2[:], pair(bi_, s + 1), di0, pair(br_, s + 2), OP.mult, OP.add)
            nc.vector.scalar_tensor_tensor(
                pair(br_, s), pair(br_, s + 1), drs, tmp2[:], OP.mult, OP.subtract)
            nc.vector.scalar_tensor_tensor(
                tmp2[:], pair(br_, s + 1), di0, pair(bi_, s + 2), OP.mult, OP.subtract)
            nc.vector.scalar_tensor_tensor(
                pair(bi_, s), pair(bi_, s + 1), drs, tmp2[:], OP.mult, OP.add)

        # ============ cross-block scan on (2, 64) layout ============
        # bounce the 8 block-matrix entries per direction to (2,64)
        # fwd block mat [[A,B],[C,D]] = [[a_31,b_31],[a_30,b_30]] (cols K+1, K)
        # bwd block mat = [[a_0,b_0],[a_1,b_1]] (cols 0, 1)
        fwd_cols = [
            fr[:, W2 - 1 + 0:W2], fi[:, W2 - 1:W2],                    # A
            fr[:, 2 * W2 - 1:2 * W2], fi[:, 2 * W2 - 1:2 * W2],        # B
            fr[:, W2 - 2:W2 - 1], fi[:, W2 - 2:W2 - 1],                # C
            fr[:, 2 * W2 - 2:2 * W2 - 1], fi[:, 2 * W2 - 2:2 * W2 - 1],  # D
        ]
        bwd_cols = [
            br_[:, 0:1], bi_[:, 0:1],
            br_[:, W2:W2 + 1], bi_[:, W2:W2 + 1],
            br_[:, 1:2], bi_[:, 1:2],
            br_[:, W2 + 1:W2 + 2], bi_[:, W2 + 1:W2 + 2],
        ]
        for i, c in enumerate(fwd_cols + bwd_cols):
            nc.sync.dma_start(cbd[i], c)

        def cross_scan(base, reverse):
            """Scan (2,64) block matrices; returns carry-into-block (2,64)
            tiles (Lr, Li)."""
            M = [scan_p.tile([2, NBLK], fp32, tag=f"cm{base}{i}", name=f"cm{base}{i}") for i in range(8)]
            for i in range(8):
                nc.sync.dma_start(M[i][:], cbd[base + i].rearrange("(r j) -> r j", r=2))
            # normalize by max entry magnitude
            t0 = scan_p.tile([2, NBLK], fp32, tag=f"cn0{base}")
            t1 = scan_p.tile([2, NBLK], fp32, tag=f"cn1{base}")
            mx = scan_p.tile([2, NBLK], fp32, tag=f"cmx{base}")
            for i in range(4):
                nc.vector.tensor_mul(t0[:], M[2 * i][:], M[2 * i][:])
                nc.vector.tensor_mul(t1[:], M[2 * i + 1][:], M[2 * i + 1][:])
                nc.vector.tensor_add(t0[:], t0[:], t1[:])
                if i == 0:
                    nc.vector.tensor_copy(mx[:], t0[:])
                else:
                    nc.vector.tensor_max(mx[:], mx[:], t0[:])
            nc.vector.reciprocal(mx[:], mx[:])
            nc.scalar.sqrt(mx[:], mx[:])
            for i in range(8):
                nc.vector.tensor_mul(M[i][:], M[i][:], mx[:])

            # view blocks as (2, SUP, 8): within-super sequential prefix
            def v3(t):
                return t.rearrange("r (u t) -> r u t", t=NBLK // SUP)

            P = [scan_p.tile([2, NBLK], fp32, tag=f"cp{base}{i}", name=f"cp{base}{i}") for i in range(8)]
            for i in range(8):
                nc.vector.tensor_copy(P[i][:], M[i][:])
            pr2 = [scan_p.tile([2, SUP], fp32, tag=f"pr2{base}{i}", name=f"pr2{base}{i}") for i in range(4)]
            idx = range(1, NBLK // SUP) if not reverse else range(NBLK // SUP - 2, -1, -1)
            for t in idx:
                tp = t - 1 if not reverse else t + 1
                # X = M[:,t] (2x2 cplx), Y = P[:,tp];  P[:,t] = X*Y
                Xa_r, Xa_i, Xb_r, Xb_i, Xc_r, Xc_i, Xd_r, Xd_i = (
                    v3(M[i])[:, :, t] for i in range(8))
                Ya_r, Ya_i, Yb_r, Yb_i, Yc_r, Yc_i, Yd_r, Yd_i = (
                    v3(P[i])[:, :, tp] for i in range(8))
                outs = [v3(P[i])[:, :, t] for i in range(8)]

                def cmul_acc(dst_r, dst_i, pr, pi, qr, qi, first):
                    # dst += p*q (complex); first -> overwrite
                    nc.vector.tensor_mul(pr2[0][:], pr, qr)
                    nc.vector.tensor_mul(pr2[1][:], pi, qi)
                    nc.vector.tensor_sub(pr2[0][:], pr2[0][:], pr2[1][:])
                    nc.vector.tensor_mul(pr2[2][:], pr, qi)
                    nc.vector.tensor_mul(pr2[3][:], pi, qr)
                    nc.vector.tensor_add(pr2[2][:], pr2[2][:], pr2[3][:])
                    if first:
                        nc.vector.tensor_copy(dst_r, pr2[0][:])
                        nc.vector.tensor_copy(dst_i, pr2[2][:])
                    else:
                        nc.vector.tensor_add(dst_r, dst_r, pr2[0][:])
                        nc.vector.tensor_add(dst_i, dst_i, pr2[2][:])

                # new_a = Xa*Ya + Xb*Yc ; new_b = Xa*Yb + Xb*Yd
                # new_c = Xc*Ya + Xd*Yc ; new_d = Xc*Yb + Xd*Yd
                cmul_acc(outs[0], outs[1], Xa_r, Xa_i, Ya_r, Ya_i, True)
                cmul_acc(outs[0], outs[1], Xb_r, Xb_i, Yc_r, Yc_i, False)
                cmul_acc(outs[2], outs[3], Xa_r, Xa_i, Yb_r, Yb_i, True)
                cmul_acc(outs[2], outs[3], Xb_r, Xb_i, Yd_r, Yd_i, False)
                cmul_acc(outs[4], outs[5], Xc_r, Xc_i, Ya_r, Ya_i, True)
                cmul_acc(outs[4], outs[5], Xd_r, Xd_i, Yc_r, Yc_i, False)
                cmul_acc(outs[6], outs[7], Xc_r, Xc_i, Yb_r, Yb_i, True)
                cmul_acc(outs[6], outs[7], Xd_r, Xd_i, Yd_r, Yd_i, False)

            # serial cross-super scan: carry (2,1), SC tile (2, SUP)
            SC_r = scan_p.tile([2, SUP], fp32, tag=f"scr{base}")
            SC_i = scan_p.tile([2, SUP], fp32, tag=f"sci{base}")
            car = scan_p.tile([2, 8], fp32, tag=f"car{base}")  # [Lr,Li,nr,ni,dr,di,m,inv]
            nc.gpsimd.memset(car[:, 0:1], 1.0)
            nc.gpsimd.memset(car[:, 1:2], 0.0)
            sidx = range(SUP) if not reverse else range(SUP - 1, -1, -1)
            last_t = (NBLK // SUP - 1) if not reverse else 0
            for u in sidx:
                nc.vector.tensor_copy(SC_r[:, u:u + 1], car[:, 0:1])
                nc.vector.tensor_copy(SC_i[:, u:u + 1], car[:, 1:2])
                Pa = [v3(P[i])[:, u:u + 1, last_t] for i in range(8)]
                Lr, Li = car[:, 0:1], car[:, 1:2]
                # num = A*L + B ; den = C*L + D
                nc.vector.tensor_mul(car[:, 2:3], Pa[0], Lr)
                nc.vector.tensor_mul(car[:, 6:7], Pa[1], Li)
                nc.vector.tensor_sub(car[:, 2:3], car[:, 2:3], car[:, 6:7])
                nc.vector.tensor_add(car[:, 2:3], car[:, 2:3], Pa[2])
                nc.vector.tensor_mul(car[:, 3:4], Pa[0], Li)
                nc.vector.tensor_mul(car[:, 6:7], Pa[1], Lr)
                nc.vector.tensor_add(car[:, 3:4], car[:, 3:4], car[:, 6:7])
                nc.vector.tensor_add(car[:, 3:4], car[:, 3:4], Pa[3])
                nc.vector.tensor_mul(car[:, 4:5], Pa[4], Lr)
                nc.vector.tensor_mul(car[:, 6:7], Pa[5], Li)
                nc.vector.tensor_sub(car[:, 4:5], car[:, 4:5], car[:, 6:7])
                nc.vector.tensor_add(car[:, 4:5], car[:, 4:5], Pa[6])
                nc.vector.tensor_mul(car[:, 5:6], Pa[4], Li)
                nc.vector.tensor_mul(car[:, 6:7], Pa[5], Lr)
                nc.vector.tensor_add(car[:, 5:6], car[:, 5:6], car[:, 6:7])
                nc.vector.tensor_add(car[:, 5:6], car[:, 5:6], Pa[7])
                # L = num * conj(den) / |den|^2
                nc.vector.tensor_mul(car[:, 6:7], car[:, 4:5], car[:, 4:5])
                nc.vector.tensor_mul(car[:, 7:8], car[:, 5:6], car[:, 5:6])
                nc.vector.tensor_add(car[:, 6:7], car[:, 6:7], car[:, 7:8])
                nc.vector.reciprocal(car[:, 6:7], car[:, 6:7])
                nc.vector.tensor_mul(car[:, 0:1], car[:, 2:3], car[:, 4:5])
                nc.vector.tensor_mul(car[:, 7:8], car[:, 3:4], car[:, 5:6])
                nc.vector.tensor_add(car[:, 0:1], car[:, 0:1], car[:, 7:8])
                nc.vector.tensor_mul(car[:, 0:1], car[:, 0:1], car[:, 6:7])
                nc.vector.tensor_mul(car[:, 7:8], car[:, 2:3], car[:, 5:6])
                nc.vector.tensor_mul(car[:, 2:3], car[:, 3:4], car[:, 4:5])
                nc.vector.tensor_sub(car[:, 1:2], car[:, 2:3], car[:, 7:8])
                nc.vector.tensor_mul(car[:, 1:2], car[:, 1:2], car[:, 6:7])

            # vectorized Mobius of all prefixes with broadcast super-carries
            SCb_r = scan_p.tile([2, NBLK], fp32, tag=f"scbr{base}")
            SCb_i = scan_p.tile([2, NBLK], fp32, tag=f"scbi{base}")
            for t in range(NBLK // SUP):
                nc.vector.tensor_copy(v3(SCb_r)[:, :, t], SC_r[:])
                nc.vector.tensor_copy(v3(SCb_i)[:, :, t], SC_i[:])
            nr = scan_p.tile([2, NBLK], fp32, tag=f"nr{base}")
            ni = scan_p.tile([2, NBLK], fp32, tag=f"ni{base}")
            dr_ = scan_p.tile([2, NBLK], fp32, tag=f"dr{base}")
            di_ = scan_p.tile([2, NBLK], fp32, tag=f"di{base}")
            nc.vector.tensor_mul(nr[:], P[0][:], SCb_r[:])
            nc.vector.tensor_mul(t0[:], P[1][:], SCb_i[:])
            nc.vector.tensor_sub(nr[:], nr[:], t0[:])
            nc.vector.tensor_add(nr[:], nr[:], P[2][:])
            nc.vector.tensor_mul(ni[:], P[0][:], SCb_i[:])
            nc.vector.tensor_mul(t0[:], P[1][:], SCb_r[:])
            nc.vector.tensor_add(ni[:], ni[:], t0[:])
            nc.vector.tensor_add(ni[:], ni[:], P[3][:])
            nc.vector.tensor_mul(dr_[:], P[4][:], SCb_r[:])
            nc.vector.tensor_mul(t0[:], P[5][:], SCb_i[:])
            nc.vector.tensor_sub(dr_[:], dr_[:], t0[:])
            nc.vector.tensor_add(dr_[:], dr_[:], P[6][:])
            nc.vector.tensor_mul(di_[:], P[4][:], SCb_i[:])
            nc.vector.tensor_mul(t0[:], P[5][:], SCb_r[:])
            nc.vector.tensor_add(di_[:], di_[:], t0[:])
            nc.vector.tensor_add(di_[:], di_[:], P[7][:])
            nc.vector.tensor_mul(t0[:], dr_[:], dr_[:])
            nc.vector.tensor_mul(t1[:], di_[:], di_[:])
            nc.vector.tensor_add(t0[:], t0[:], t1[:])
            nc.vector.reciprocal(t0[:], t0[:])
            MA_r = scan_p.tile([2, NBLK], fp32, tag=f"mar{base}")
            MA_i = scan_p.tile([2, NBLK], fp32, tag=f"mai{base}")
            nc.vector.tensor_mul(MA_r[:], nr[:], dr_[:])
            nc.vector.tensor_mul(t1[:], ni[:], di_[:])
            nc.vector.tensor_add(MA_r[:], MA_r[:], t1[:])
            nc.vector.tensor_mul(MA_r[:], MA_r[:], t0[:])
            nc.vector.tensor_mul(MA_i[:], ni[:], dr_[:])
            nc.vector.tensor_mul(t1[:], nr[:], di_[:])
            nc.vector.tensor_sub(MA_i[:], MA_i[:], t1[:])
            nc.vector.tensor_mul(MA_i[:], MA_i[:], t0[:])
            # carry-into-block: shift within super + overwrite first col
            Cr = scan_p.tile([2, NBLK], fp32, tag=f"cr{base}")
            Ci = scan_p.tile([2, NBLK], fp32, tag=f"ci{base}")
            if not reverse:
                nc.vector.tensor_copy(Cr[:, 1:], MA_r[:, :NBLK - 1])
                nc.vector.tensor_copy(Ci[:, 1:], MA_i[:, :NBLK - 1])
                nc.vector.tensor_copy(v3(Cr)[:, :, 0], SC_r[:])
                nc.vector.tensor_copy(v3(Ci)[:, :, 0], SC_i[:])
            else:
                nc.vector.tensor_copy(Cr[:, :NBLK - 1], MA_r[:, 1:])
                nc.vector.tensor_copy(Ci[:, :NBLK - 1], MA_i[:, 1:])
                nc.vector.tensor_copy(v3(Cr)[:, :, NBLK // SUP - 1], SC_r[:])
                nc.vector.tensor_copy(v3(Ci)[:, :, NBLK // SUP - 1], SC_i[:])
            return Cr, Ci

        Lf_r, Lf_i = cross_scan(0, reverse=False)
        Rb_r, Rb_i = cross_scan(8, reverse=True)

        # bounce carries to (128,1) lane layout
        nc.sync.dma_start(lcd[0], Lf_r[:])
        nc.sync.dma_start(lcd[1], Lf_i[:])
        nc.sync.dma_start(lcd[2], Rb_r[:])
        nc.sync.dma_start(lcd[3], Rb_i[:])
        LinR = scan_p.tile([128, 1], fp32, tag="LinR")
        LinI = scan_p.tile([128, 1], fp32, tag="LinI")
        RinR = scan_p.tile([128, 1], fp32, tag="RinR")
        RinI = scan_p.tile([128, 1], fp32, tag="RinI")
        nc.sync.dma_start(LinR[:], lcd[0].rearrange("(p c) -> p c", c=1))
        nc.sync.dma_start(LinI[:], lcd[1].rearrange("(p c) -> p c", c=1))
        nc.sync.dma_start(RinR[:], lcd[2].rearrange("(p c) -> p c", c=1))
        nc.sync.dma_start(RinI[:], lcd[3].rearrange("(p c) -> p c", c=1))

        # ============ application: L, R, G (all (128, KS)) ============
        ap_p = scan_p

        def mobius_apply(ar_lo, ai_lo, br_lo, bi_lo, ar_hi, ai_hi, br_hi, bi_hi,
                         Kr, Ki, tag):
            # hi = numerator coeff cols, lo = denominator coeff cols
            X1 = ap_p.tile([128, KS], fp32, tag=f"x1{tag}")
            X2 = ap_p.tile([128, KS], fp32, tag=f"x2{tag}")
            numr = ap_p.tile([128, KS], fp32, tag=f"numr{tag}")
            numi = ap_p.tile([128, KS], fp32, tag=f"numi{tag}")
            denr = ap_p.tile([128, KS], fp32, tag=f"denr{tag}")
            deni = ap_p.tile([128, KS], fp32, tag=f"deni{tag}")
            nc.vector.scalar_tensor_tensor(X1[:], ar_hi, Kr, br_hi, OP.mult, OP.add)
            nc.vector.tensor_scalar_mul(X2[:], ai_hi, Ki)
            nc.vector.tensor_sub(numr[:], X1[:], X2[:])
            nc.vector.scalar_tensor_tensor(X1[:], ai_hi, Kr, bi_hi, OP.mult, OP.add)
            nc.vector.tensor_scalar_mul(X2[:], ar_hi, Ki)
            nc.vector.tensor_add(numi[:], X1[:], X2[:])
            nc.vector.scalar_tensor_tensor(X1[:], ar_lo, Kr, br_lo, OP.mult, OP.add)
            nc.vector.tensor_scalar_mul(X2[:], ai_lo, Ki)
            nc.vector.tensor_sub(denr[:], X1[:], X2[:])
            nc.vector.scalar_tensor_tensor(X1[:], ai_lo, Kr, bi_lo, OP.mult, OP.add)
            nc.vector.tensor_scalar_mul(X2[:], ar_lo, Ki)
            nc.vector.tensor_add(deni[:], X1[:], X2[:])
            nc.vector.tensor_mul(X1[:], denr[:], denr[:])
            nc.vector.tensor_mul(X2[:], deni[:], deni[:])
            nc.vector.tensor_add(X1[:], X1[:], X2[:])
            nc.vector.reciprocal(X1[:], X1[:])
            Lr = ap_p.tile([128, KS], fp32, tag=f"lr{tag}")
            Li = ap_p.tile([128, KS], fp32, tag=f"li{tag}")
            nc.vector.tensor_mul(Lr[:], numr[:], denr[:])
            nc.vector.tensor_mul(X2[:], numi[:], deni[:])
            nc.vector.tensor_add(Lr[:], Lr[:], X2[:])
            nc.vector.tensor_mul(Lr[:], Lr[:], X1[:])
            nc.vector.tensor_mul(Li[:], numi[:], denr[:])
            nc.vector.tensor_mul(X2[:], numr[:], deni[:])
            nc.vector.tensor_sub(Li[:], Li[:], X2[:])
            nc.vector.tensor_mul(Li[:], Li[:], X1[:])
            return Lr, Li

        Lr, Li = mobius_apply(
            fr[:, 1:W2 - 1], fi[:, 1:W2 - 1], fr[:, W2 + 1:2 * W2 - 1], fi[:, W2 + 1:2 * W2 - 1],
            fr[:, 2:W2], fi[:, 2:W2], fr[:, W2 + 2:2 * W2], fi[:, W2 + 2:2 * W2],
            LinR[:], LinI[:], "L")
        Rr, Ri = mobius_apply(
            br_[:, 1:W2 - 1], bi_[:, 1:W2 - 1], br_[:, W2 + 1:2 * W2 - 1], bi_[:, W2 + 1:2 * W2 - 1],
            br_[:, 0:KS], bi_[:, 0:KS], br_[:, W2:W2 + KS], bi_[:, W2:W2 + KS],
            RinR[:], RinI[:], "R")

        # G = 1/(L + R - d) ; clip; cast bf16; bounce to chunk-major
        wr = ap_p.tile([128, KS], fp32, tag="wr")
        wi = ap_p.tile([128, KS], fp32, tag="wi")
        gt0 = ap_p.tile([128, KS], fp32, tag="gt0")
        nc.vector.tensor_add(wr[:], Lr[:], Rr[:])
        nc.vector.tensor_sub(wr[:], wr[:], he[:])
        nc.vector.tensor_add(wi[:], Li[:], Ri[:])
        nc.vector.tensor_sub(wi[:], wi[:], dim_s[:])
        wr2 = ap_p.tile([128, KS], fp32, tag="wr2")
        nc.vector.tensor_mul(gt0[:], wr[:], wr[:])
        nc.vector.tensor_mul(wr2[:], wi[:], wi[:])
        nc.vector.tensor_add(gt0[:], gt0[:], wr2[:])
        nc.vector.reciprocal(gt0[:], gt0[:])
        grt = ap_p.tile([128, KS], bfl, tag="grt")
        git = ap_p.tile([128, KS], bfl, tag="git")
        nc.vector.tensor_mul(wr[:], wr[:], gt0[:])
        nc.vector.tensor_scalar(grt[:], wr[:], FCLAMP, -FCLAMP, OP.min, OP.max)
        nc.vector.tensor_mul(wi[:], wi[:], gt0[:])
        nc.vector.tensor_scalar_mul(wi[:], wi[:], -1.0)
        nc.vector.tensor_scalar(git[:], wi[:], FCLAMP, -FCLAMP, OP.min, OP.max)
        nc.sync.dma_start(grd[:], grt[:])
        nc.sync.dma_start(gid[:], git[:])
        GrT = ap_p.tile([128, KS], bfl, tag="GrT")
        GiT = ap_p.tile([128, KS], bfl, tag="GiT")
        nc.sync.dma_start(GrT[:], grd.rearrange("(k b) s -> (b s) k", b=4))
        nc.sync.dma_start(GiT[:], gid.rearrange("(k b) s -> (b s) k", b=4))

        # ============ gather G to slots: on-device one-hot matmuls ============
        rhs_aug = big_p.tile([2, CAP], bfl, tag="rhsaug")
        pgr = [ps_g.tile([1, w], fp32, tag=f"pgr{j}", name=f"pgr{j}") for j, (o, w) in enumerate(NCH)]
        pgi = [ps_g.tile([1, w], fp32, tag=f"pgi{j}", name=f"pgi{j}") for j, (o, w) in enumerate(NCH)]
        for k in range(NT // 128):
            # one-hot chunk: pt[p, s] = (tokb[s] - iota[p] == 128k)
            pt = p_p.tile([128, CAP], bfl, tag="pt")
            nc.vector.tensor_scalar(pt[:], tokb_s[:], io_s[:], float(128 * k),
                                    OP.subtract, OP.is_equal)
            for j, (o, w) in enumerate(NCH):
                nc.tensor.matmul(pgr[j], GrT[:, k:k + 1], pt[:, o:o + w],
                                 start=(k == 0), stop=(k == NT // 128 - 1))
                nc.tensor.matmul(pgi[j], GiT[:, k:k + 1], pt[:, o:o + w],
                                 start=(k == 0), stop=(k == NT // 128 - 1))
        gi_sb = big_p.tile([1, CAP], bfl, tag="gisb")
        for j, (o, w) in enumerate(NCH):
            nc.scalar.copy(rhs_aug[0:1, o:o + w], pgr[j][:])
            nc.scalar.copy(gi_sb[:, o:o + w], pgi[j][:])
        nc.sync.dma_start(rhs_aug[1:2, :], gi_sb[:])

        # ============ MM1: hT = gelu(w1 @ xgT + b1) ============
        xg_s = big_p.tile([128, DCH * CAP], bfl, tag="xgs")
        for k in range(DCH):
            nc.sync.dma_start(xg_s[:, CAP * k:CAP * (k + 1)],
                              xgt[128 * k:128 * (k + 1), :])
        hT = big_p.tile([128, FCH * CAP], bfl, tag="hT")
        for f in range(FCH):
            pss = [ps_mm.tile([128, w], fp32, tag=f"psmm{j}", name=f"ps1f{f}j{j}") for j, (o, w) in enumerate(NCH)]
            w1f = w_p.tile([128, DCH * 128], bfl, tag="w1f", name=f"w1f{f}")
            nc.sync.dma_start(
                w1f[:],
                w1g.rearrange("(k p) q -> p k q", p=128)[:, :, 128 * f:128 * (f + 1)])
            for k in range(DCH):
                for j, (o, w) in enumerate(NCH):
                    nc.tensor.matmul(pss[j][:], w1f[:, 128 * k:128 * (k + 1)],
                                     xg_s[:, CAP * k + o:CAP * k + o + w],
                                     start=(k == 0), stop=(k == DCH - 1))
            for j, (o, w) in enumerate(NCH):
                # gelu (tanh approx) computed explicitly across engines
                xb = xin_p.tile([128, w], fp32, tag=f"gxb{j}", name=f"gxb{f}{j}")
                sq = xin_p.tile([128, w], fp32, tag=f"gsq{j}", name=f"gsq{f}{j}")
                tt = xin_p.tile([128, w], fp32, tag=f"gtt{j}", name=f"gtt{f}{j}")
                nc.scalar.activation(xb[:], pss[j][:], AF.Identity,
                                     bias=b1_s[:, f:f + 1])
                nc.gpsimd.tensor_mul(sq[:], xb[:], xb[:])
                nc.gpsimd.tensor_mul(sq[:], sq[:], xb[:])
                nc.vector.scalar_tensor_tensor(sq[:], sq[:], 0.044715, xb[:],
                                               OP.mult, OP.add)
                nc.scalar.activation(tt[:], sq[:], AF.Tanh, scale=0.7978845608028654)
                nc.vector.tensor_scalar(tt[:], tt[:], 1.0, 0.5, OP.add, OP.mult)
                nc.gpsimd.tensor_mul(hT[:, CAP * f + o:CAP * f + o + w],
                                     tt[:], xb[:])

        # ============ MM2: out = w2 @ hT + spec + bias ============
        for dch in range(DCH):
            pso = [ps_mm.tile([128, w], fp32, tag=f"psmm{j}", name=f"ps2d{dch}j{j}") for j, (o, w) in enumerate(NCH)]
            w2f = w_p.tile([128, FCH * 128], bfl, tag="w2f", name=f"w2f{dch}")
            nc.sync.dma_start(
                w2f[:],
                w2g.rearrange("(k p) q -> p k q", p=128)[:, :, 128 * dch:128 * (dch + 1)])
            for f in range(FCH):
                for j, (o, w) in enumerate(NCH):
                    nc.tensor.matmul(pso[j][:], w2f[:, 128 * f:128 * (f + 1)],
                                     hT[:, CAP * f + o:CAP * f + o + w],
                                     start=(f == 0), stop=False)
            for j, (o, w) in enumerate(NCH):
                nc.tensor.matmul(pso[j][:], waug_s[:, 128 * dch:128 * (dch + 1)],
                                 rhs_aug[:, o:o + w], start=False, stop=True)
            ot = xin_p.tile([128, CAP], fp16, tag="ot")
            for j, (o, w) in enumerate(NCH):
                nc.scalar.activation(ot[:, o:o + w], pso[j][:],
                                     AF.Identity, bias=ball_s[:, dch:dch + 1])
            nc.sync.dma_start(outg[128 * dch:128 * (dch + 1), :], ot[:])

    nc.compile()
    return nc


def _get_program():
    if "main" not in _PROG_CACHE:
        _PROG_CACHE["main"] = _build_program()
    return _PROG_CACHE["main"]


def _np(a):
    return np.asarray(a)


def kernel(**inputs) -> np.ndarray:
    from concourse.bass_utils import run_bass_kernel_spmd

    x = _np(inputs["x"]).astype(np.float32)
    v_w = _np(inputs["v_w"]).astype(np.float32)
    v_b = float(_np(inputs["v_b"]))
    gate_w = _np(inputs["gate_w"]).astype(np.float32)
    gate_b = _np(inputs["gate_b"]).astype(np.float32)
    w1 = _np(inputs["w1"]).astype(np.float32)
    b1 = _np(inputs["b1"]).astype(np.float32)
    w2 = _np(inputs["w2"]).astype(np.float32)
    b2 = _np(inputs["b2"]).astype(np.float32)
    out_w = _np(inputs["out_w"]).astype(np.float32)
    out_b = _np(inputs["out_b"]).astype(np.float32)
    bk_scale = _np(inputs["bk_scale"]).astype(np.float32)
    eps_p = float(_np(inputs["epsilon_param"]))
    gamma = float(_np(inputs["gamma"]))

    x2 = x.reshape(NT, D)
    logits = x2 @ gate_w.T + gate_b
    eidx = np.argmax(logits, axis=-1)

    counts = np.bincount(eidx, minlength=E)
    if counts.max() > 2 * CAP:
        return _host_fallback(x, v_w, v_b, gate_w, gate_b, w1, b1, w2, b2,
                              out_w, out_b, bk_scale, eps_p, gamma)

    eps = float(np.log1p(np.exp(eps_p))) + 1e-6
    dim_val = -(eps + gamma)

    # potential / scan input, computed host-side (tiny matvec)
    v2 = np.clip(x2 @ v_w + v_b, -V_MAX, V_MAX).astype(np.float32) - 2.0

    lanes = np.arange(128)
    common = {
        "he": v2.reshape(128, KS),
        "dimt": np.full((128, KS), dim_val, np.float32),
        "cfirst": (lanes % NBLK != 0).astype(np.float32).reshape(128, 1),
        "clast": (lanes % NBLK != NBLK - 1).astype(np.float32).reshape(128, 1),
        "iotac": lanes.astype(np.float32).reshape(128, 1),
    }
    Wp = (bk_scale[:, None] * out_w).astype(np.float32)  # (D, 2)
    waug_arr = np.ascontiguousarray(Wp.T).astype(bf16)

    in_maps = []
    slot_tok = []  # per core: (token_indices, n_real)
    for c in range(NC):
        e, half = c // 2, c % 2
        toks = np.where(eidx == e)[0][half * CAP:(half + 1) * CAP]
        n = len(toks)
        xg = np.zeros((CAP, D), np.float32)
        xg[:n] = x2[toks]
        tokrow = np.full(CAP, -1.0, np.float32)
        tokrow[:n] = toks.astype(np.float32)
        ball = b2[e] + bk_scale * out_b
        w1t = w1[e].T  # (D, F)
        w2t = w2[e].T  # (F, D)
        m = dict(common)
        m.update({
            "xgt": np.ascontiguousarray(xg.T).astype(bf16),
            "w1h": np.ascontiguousarray(w1t[half * (D // 2):(half + 1) * (D // 2), :]).astype(bf16),
            "w2h": np.ascontiguousarray(w2t[half * FH:(half + 1) * FH, :]).astype(bf16),
            "waug": waug_arr,
            "tokb": np.broadcast_to(tokrow, (128, CAP)).copy(),
            "b1t": np.ascontiguousarray(b1[e].reshape(F // 128, 128).T).astype(np.float32),
            "ballt": np.ascontiguousarray(ball.reshape(D // 128, 128).T).astype(np.float32),
        })
        in_maps.append(m)
        slot_tok.append((toks, n))

    nc = _get_program()
    global _LAST_IN_MAPS
    _LAST_IN_MAPS = in_maps
    res = run_bass_kernel_spmd(nc, in_maps, list(range(NC))).results

    out2 = np.zeros((NT, D), np.float32)
    for c in range(NC):
        toks, n = slot_tok[c]
        out2[toks] = res[c]["outg"][:, :n].T.astype(np.float32)
    return out2.reshape(B, N, D)


def _host_fallback(x, v_w, v_b, gate_w, gate_b, w1, b1, w2, b2,
                   out_w, out_b, bk_scale, eps_p, gamma):
    x2 = x.reshape(NT, D)
    v = np.clip(x2 @ v_w + v_b, -V_MAX, V_MAX).reshape(B, N)
    eps = float(np.log1p(np.exp(eps_p))) + 1e-6
    d = (v - 2.0).astype(np.complex64) - 1j * (eps + gamma)
    dT = d.T
    c = np.concatenate([np.zeros((1, B)), np.ones((N - 1, B))], 0)
    Lv = np.zeros((N, B), np.complex64)
    carry = np.ones(B, np.complex64)
    for i in range(N):
        carry = dT[i] - c[i] / carry
        Lv[i] = carry
    Rr = np.zeros((N, B), np.complex64)
    carry = np.ones(B, np.complex64)
    for i in range(N):
        carry = dT[::-1][i] - c[i] / carry
        Rr[i] = carry
    G = (1.0 / (Lv + Rr[::-1] - dT)).T
    feats = np.clip(np.stack([G.real, G.imag], -1), -FCLAMP, FCLAMP)
    spec = feats @ out_w.T + out_b
    logits = x2 @ gate_w.T + gate_b
    eidx = np.argmax(logits, axis=-1)
    out2 = np.zeros((NT, D), np.float32)
    for e in range(E):
        sl = eidx == e
        hp = x2[sl] @ w1[e].T + b1[e]
        h = 0.5 * hp * (1 + np.tanh(np.sqrt(2 / np.pi) * (hp + 0.044715 * hp ** 3)))
        out2[sl] = h @ w2[e].T + b2[e]
    out = out2.reshape(B, N, D) + bk_scale * spec
    return out.astype(np.float32)
